# revision 3
# baseline (speedup 1.0000x reference)
"""Trainium2 Bass kernel for nn_ExemplarNoAttention (retrieval_knn).

logits[b,c] = log(eps + sum_{e: label[e]==c} exp(-beta * ||x_b - E_e||^2))

Strategy: shard the exemplar bank Ne=50000 across 8 NeuronCores (6250/core).
Host-side prep (cheap O(Ne*d) layout work): reorder exemplars so each core's
slab is grouped by class with per-class segments padded to a fixed size that
is identical on every core (SPMD: one program, per-core data). The padded
slots are "-inf" exemplars whose similarity is exactly 0.

Device (per core):
  GEMM (TensorE, bf16):  psum[b, e] = 2*beta*<x_b, E_e> - beta*||E_e||^2
     via K=65 contraction: rows 0..63 = features, row 64 = (1, -beta*e2).
  ScalarE:  sims[b,e] = Exp(psum + bias_b), bias_b = -beta*||x_b||^2  (bf16)
  VectorE:  per-class segment sums via tensor_scalar(accum_out) at 4x mode
  AllReduce of partial class sums (1024 x 10 f32) across the 8 cores
  ScalarE:  logits = Ln(class_sims + eps);  DMA out (identical on all cores)
"""

import os
import numpy as np
import ml_dtypes

NUM_CLASSES = 10
GAMMA = 1.0
EPS = 1e-12
N_CORES = 8
B = 1024
D = 64
NE = 50000
BT = 128            # batch rows per psum tile (output partitions)
NBT = B // BT       # 8 batch tiles
SEG_ALIGN = 32      # per-class segment padding granularity
CHUNK = 512         # matmul moving-operand chunk (one PSUM bank)
WIN = 2048          # psum window drained by one activation (4 banks)

# results of the last hardware run (for test harness introspection)
LAST_EXEC_NS = None
LAST_RESULTS = None
TRACE = bool(int(os.environ.get("KERNEL_TRACE", "0")))
TRACE_DIR = os.environ.get("KERNEL_TRACE_DIR", "")


def _host_prep(x, exemplars, exemplar_labels, beta_raw):
    x = np.asarray(x, dtype=np.float32)
    E = np.asarray(exemplars, dtype=np.float32)
    labels = np.asarray(exemplar_labels).astype(np.int64)
    beta = float(np.logaddexp(0.0, np.float64(beta_raw.reshape(-1)[0])))

    # per-(core, class) index lists, balanced within each class
    per_core_idx = [[None] * NUM_CLASSES for _ in range(N_CORES)]
    max_cc = np.zeros(NUM_CLASSES, dtype=np.int64)
    for c in range(NUM_CLASSES):
        idx_c = np.nonzero(labels == c)[0]
        n = len(idx_c)
        base, rem = divmod(n, N_CORES)
        sizes = [base + (1 if i < rem else 0) for i in range(N_CORES)]
        off = 0
        for i in range(N_CORES):
            per_core_idx[i][c] = idx_c[off:off + sizes[i]]
            off += sizes[i]
        max_cc[c] = max(sizes) if n else 0

    seg_sizes = [int(-(-m // SEG_ALIGN) * SEG_ALIGN) for m in max_cc]
    seg_offs = np.concatenate([[0], np.cumsum(seg_sizes)]).astype(np.int64)
    e_used = int(seg_offs[-1])
    e_pad = int(-(-e_used // CHUNK) * CHUNK)

    # build per-core augmented transposed exemplar slabs (K=65, e_pad), bf16
    e2 = (E.astype(np.float64) ** 2).sum(axis=1)
    ea_cores = []
    for i in range(N_CORES):
        ea = np.zeros((D + 1, e_pad), dtype=np.float32)
        ea[D, :] = -1.0e38  # padded slots: exp() == 0
        for c in range(NUM_CLASSES):
            idx = per_core_idx[i][c]
            o = int(seg_offs[c])
            if len(idx):
                ea[:D, o:o + len(idx)] = (2.0 * beta) * E[idx].T
                ea[D, o:o + len(idx)] = (-beta * e2[idx]).astype(np.float32)
        ea_cores.append(ea.astype(ml_dtypes.bfloat16))

    # augmented x^T (stationary operand), bf16
    xa = np.ones((D + 1, B), dtype=np.float32)
    xa[:D, :] = x.T
    xa = xa.astype(ml_dtypes.bfloat16)

    # per-partition activation bias: -beta*||x_b||^2, laid out (BT, NBT)
    x2 = (x.astype(np.float64) ** 2).sum(axis=1)
    bias = (-beta * x2).astype(np.float32).reshape(NBT, BT).T.copy()

    return ea_cores, xa, bias, seg_offs, seg_sizes, e_pad


def _build_program(seg_offs, seg_sizes, e_pad):
    from contextlib import ExitStack
    import concourse.bass as bass
    import concourse.tile as tile
    from concourse import bacc, mybir

    f32 = mybir.dt.float32
    bf16 = mybir.dt.bfloat16

    nc = bacc.Bacc(
        "TRN2",
        target_bir_lowering=False,
        debug=False,
        enable_asserts=False,
        num_devices=N_CORES,
    )

    ea_d = nc.dram_tensor("ea", [D + 1, e_pad], bf16, kind="ExternalInput").ap()
    xa_d = nc.dram_tensor("xa", [D + 1, B], bf16, kind="ExternalInput").ap()
    bias_d = nc.dram_tensor("biasx", [BT, NBT], f32, kind="ExternalInput").ap()
    out_d = nc.dram_tensor("logits", [B, NUM_CLASSES], f32, kind="ExternalOutput").ap()

    max_seg = max(seg_sizes)
    # psum windows within [0, e_pad)
    wins = []
    o = 0
    while o < e_pad:
        wins.append((o, min(WIN, e_pad - o)))
        o += WIN

    with tile.TileContext(nc) as tc, ExitStack() as ctx:
        const_pool = ctx.enter_context(tc.tile_pool(name="const", bufs=1))
        psum_pool = ctx.enter_context(tc.tile_pool(name="psum", bufs=2, space="PSUM"))
        sims_pool = ctx.enter_context(tc.tile_pool(name="sims", bufs=2))
        cls_pool = ctx.enter_context(tc.tile_pool(name="cls", bufs=2))
        junk_pool = ctx.enter_context(tc.tile_pool(name="junk", bufs=2))
        res_pool = ctx.enter_context(tc.tile_pool(name="res", bufs=1))
        dram_pool = ctx.enter_context(tc.tile_pool(name="dram", bufs=1, space="DRAM"))

        # load constants
        ea_t = const_pool.tile([D + 1, e_pad], bf16, name="ea_t")
        n_load = 8
        step = -(-e_pad // n_load)
        step = -(-step // CHUNK) * CHUNK
        o = 0
        while o < e_pad:
            w = min(step, e_pad - o)
            nc.sync.dma_start(out=ea_t[:, o:o + w], in_=ea_d[:, o:o + w])
            o += w
        xa_t = const_pool.tile([D + 1, B], bf16, name="xa_t")
        nc.sync.dma_start(out=xa_t[:], in_=xa_d[:])
        bias_t = const_pool.tile([BT, NBT], f32, name="bias_t")
        nc.sync.dma_start(out=bias_t[:], in_=bias_d[:])

        bounce_in = dram_pool.tile([BT, NBT * NUM_CLASSES], f32, name="bounce_in")
        bounce_out = dram_pool.tile(
            [BT, NBT * NUM_CLASSES], f32, name="bounce_out", addr_space="Shared"
        )

        for t in range(NBT):
            sims = sims_pool.tile([BT, e_pad], bf16, tag="sims")
            lhsT = xa_t[:, t * BT:(t + 1) * BT]
            for (wo, wl) in wins:
                ps = psum_pool.tile([BT, WIN], f32, tag="ps")
                for co in range(0, wl, CHUNK):
                    cl = min(CHUNK, wl - co)
                    nc.tensor.matmul(
                        ps[:, co:co + cl],
                        lhsT=lhsT,
                        rhs=ea_t[:, wo + co:wo + co + cl],
                        start=True,
                        stop=True,
                    )
                nc.scalar.activation(
                    sims[:, wo:wo + wl],
                    ps[:, :wl],
                    mybir.ActivationFunctionType.Exp,
                    bias=bias_t[:, t:t + 1],
                    scale=1.0,
                )
            cls = cls_pool.tile([BT, NUM_CLASSES], f32, tag="cls")
            junk = junk_pool.tile([BT, max_seg], bf16, tag="junk")
            for c in range(NUM_CLASSES):
                o = int(seg_offs[c])
                s = seg_sizes[c]
                nc.vector.tensor_scalar(
                    junk[:, :s],
                    sims[:, o:o + s],
                    1.0,
                    None,
                    mybir.AluOpType.mult,
                    mybir.AluOpType.add,
                    accum_out=cls[:, c:c + 1],
                )
            nc.sync.dma_start(
                out=bounce_in[:, t * NUM_CLASSES:(t + 1) * NUM_CLASSES], in_=cls[:]
            )

        nc.gpsimd.collective_compute(
            "AllReduce",
            mybir.AluOpType.add,
            replica_groups=[list(range(N_CORES))],
            ins=[bounce_in[:].opt()],
            outs=[bounce_out[:].opt()],
        )

        res = res_pool.tile([BT, NBT * NUM_CLASSES], f32, name="res")
        nc.sync.dma_start(out=res[:], in_=bounce_out[:])
        eps_t = const_pool.tile([BT, 1], f32, name="eps_t")
        nc.vector.memset(eps_t[:], float(EPS))
        logit = res_pool.tile([BT, NBT * NUM_CLASSES], f32, name="logit")
        nc.scalar.activation(
            logit[:],
            res[:],
            mybir.ActivationFunctionType.Ln,
            bias=eps_t[:, 0:1],
            scale=1.0,
        )
        # logits dram is (B, 10) = (t p) c ; sbuf tile is [p, (t c)]
        out_ap = out_d.rearrange("(t p) c -> p t c", p=BT)
        nc.sync.dma_start(out=out_ap, in_=logit[:].rearrange("p (t c) -> p t c", t=NBT))

    nc.compile()
    return nc


def kernel(x, exemplars, exemplar_labels, beta_raw):
    global LAST_EXEC_NS, LAST_RESULTS
    from concourse.bass_utils import run_bass_kernel_spmd

    ea_cores, xa, bias, seg_offs, seg_sizes, e_pad = _host_prep(
        x, exemplars, exemplar_labels, beta_raw
    )
    nc = _build_program(seg_offs, seg_sizes, e_pad)

    in_maps = [
        {"ea": ea_cores[i], "xa": xa, "biasx": bias} for i in range(N_CORES)
    ]
    kwargs = {}
    if TRACE:
        kwargs["trace"] = True
        if TRACE_DIR:
            os.makedirs(TRACE_DIR, exist_ok=True)
            kwargs["tmpdir"] = TRACE_DIR
    ret = run_bass_kernel_spmd(nc, in_maps, list(range(N_CORES)), **kwargs)
    LAST_EXEC_NS = ret.exec_time_ns
    LAST_RESULTS = ret
    out = np.asarray(ret.results[0]["logits"], dtype=np.float32)
    return out
